# revision 11
# baseline (speedup 1.0000x reference)
"""Batched Viterbi decode (CRF layer) on 8 Trainium2 NeuronCores.

Problem: B=128, S=512, T=256 (reference.py). Per batch b:
  score_0 = start + em_0
  score_{t+1}[j] = max_i(score_t[i] + trans[i,j]) + em_{t+1}[j]
  paths via backpointer argmax; best_scores = max_j(score_{S-1} + end).

Data parallel over batch: 16 sequences per core, identical program (SPMD).

Forward (per core): scores are kept as columns hist[j(128 part), jt(2), b, t]
(pre-emission accumulator; hist[:,:,:,0] holds start_transitions). Each step:
  - emissions row-tile em_t[b,i] is transposed via PE matmul with I16 into
    column form, added (DVE) to hist columns -> scol[j,(jt,b)] = score_t.
  - scol is PE-transposed to rows [b(8-chunk), i], then per-b selector
    matmuls (lhsT = one-hot row selector) broadcast row b across all 128
    partitions into PSUM; ScalarE copies to SBUF score_bc[:, b, :].
  - VectorE: tensor_add (transT replicated over b on host) + segmented
    reduce_max(axis=X) -> next hist column block. This is the bottleneck:
    2 passes over B*S*T^2/8 elements.
Backward: path recomputed in one-hot space; C[b,i] = T[i,tag_b] +
score_t[b,i] + em_t[b,i] built by PSUM-accumulated matmuls (one-hot cols @
transT + hist cols @ I-halves + I16 @ em rows), then reduce_max + is_equal
gives the next one-hot; tag indices extracted by iota dot-product matmul.
Ties in is_equal would diverge from jnp.argmax's first-index rule, but with
continuous random inputs exact float ties do not occur.
"""

import numpy as np

B, S, T = 128, 512, 256
NCORES = 8
BC = B // NCORES   # 16 sequences per core
CH = 16            # b-chunk size for the transpose/broadcast machinery
NCH = BC // CH

_compiled = {}


def _get_addmax():
    """Register (idempotent) a fused custom DVE op:
    out = in0 + in1; accum_out = max over free dim (init -FLT_MAX).
    Replaces a tensor_add + reduce_max pair -> one DVE pass."""
    import numpy as np
    import concourse.dve_ops as dve_ops
    from concourse.dve_ops import DveOp
    from concourse.dve_spec import Spec, Src0, Src1, AluOp

    NAME = "ANT_VIT_ADD_MAX"
    for op in dve_ops.OPS:
        if op.name == NAME:
            return op

    def _ref(in0, in1, c0, c1, c2):
        out = in0.astype(np.float32) + in1
        acc = out.reshape(out.shape[0], -1).max(axis=-1, keepdims=True)
        return out, acc

    spec = Spec(body=Src0 + Src1, accum=AluOp.MAX, reference=_ref)
    op = DveOp(NAME, spec, subdim=False,
               uops_sha={"v3": "b901c41156a86946",
                         "v4": "c9dee8c65593bc95"})
    dve_ops.OPS.append(op)
    dve_ops.CUSTOM_DVE_SPECS[NAME] = spec
    dve_ops._SUB_OPCODE_FOR_NAME[NAME] = (
        dve_ops._CUSTOM_DVE_ROW_BASE + len(dve_ops.OPS) - 1)
    return op


def _build(s_len):
    import concourse.bacc as bacc
    import concourse.bass as bass
    import concourse.mybir as mybir
    import concourse.tile as tile

    f32 = mybir.dt.float32
    i32 = mybir.dt.int32
    AX = mybir.AxisListType
    OP = mybir.AluOpType

    nc = bacc.Bacc("TRN2", target_bir_lowering=False, debug=False)

    addmax = _get_addmax()
    em = nc.dram_tensor("em", [BC, s_len, T], f32, kind="ExternalInput").ap()
    transT = nc.dram_tensor("transT", [T, T], f32, kind="ExternalInput").ap()
    startc = nc.dram_tensor("startc", [T, BC], f32, kind="ExternalInput").ap()
    ileft = nc.dram_tensor("ileft", [128, T], f32, kind="ExternalInput").ap()
    iright = nc.dram_tensor("iright", [128, T], f32, kind="ExternalInput").ap()
    i16 = nc.dram_tensor("i16", [BC, BC], f32, kind="ExternalInput").ap()
    sel8 = nc.dram_tensor("sel8", [CH, BC, 128], f32, kind="ExternalInput").ap()
    iotac = nc.dram_tensor("iotac", [T, 1], f32, kind="ExternalInput").ap()

    paths_d = nc.dram_tensor("paths", [BC, s_len], i32, kind="ExternalOutput").ap()
    bs_d = nc.dram_tensor("bs", [BC, 1], f32, kind="ExternalOutput").ap()

    with tile.TileContext(nc) as tc:
        with (
            tc.tile_pool(name="const", bufs=1) as cpool,
            tc.tile_pool(name="hist", bufs=1) as hpool,
            tc.tile_pool(name="sbc", bufs=1) as sbcpool,
            tc.tile_pool(name="tmp", bufs=2) as tmppool,
            tc.tile_pool(name="ein", bufs=6) as epool,
            tc.tile_pool(name="ecol", bufs=2) as ecpool,
            tc.tile_pool(name="scol", bufs=3) as scpool,
            tc.tile_pool(name="srow", bufs=3) as srpool,
            tc.tile_pool(name="ohc", bufs=3) as ohpool,
            tc.tile_pool(name="small", bufs=4) as smallpool,
            tc.tile_pool(name="out", bufs=1) as opool,
        ):
            tt0 = cpool.tile([128, T], f32, tag="tt0")
            tt1 = cpool.tile([128, T], f32, tag="tt1")
            il_t = cpool.tile([128, T], f32, tag="il")
            ir_t = cpool.tile([128, T], f32, tag="ir")
            i16_t = cpool.tile([BC, BC], f32, tag="i16")
            sel_t = cpool.tile([CH, BC, 128], f32, tag="sel")
            io0 = cpool.tile([128, 1], f32, tag="io0")
            io1 = cpool.tile([128, 1], f32, tag="io1")
            nc.sync.dma_start(tt0[:], transT[0:128, :])
            nc.sync.dma_start(tt1[:], transT[128:256, :])
            nc.sync.dma_start(il_t[:], ileft[:])
            nc.sync.dma_start(ir_t[:], iright[:])
            nc.sync.dma_start(i16_t[:], i16[:])
            nc.sync.dma_start(sel_t[:], sel8[:])
            nc.sync.dma_start(io0[:], iotac[0:128, :])
            nc.sync.dma_start(io1[:], iotac[128:256, :])
            i128 = il_t[:, 0:128]  # identity for PE transpose

            # score history columns: hist[j(128), jt(2), b(16), t]
            hist = hpool.tile([128, 2, BC, s_len], f32, tag="hist")
            nc.sync.dma_start(hist[:, 0, :, 0], startc[0:128, :])
            nc.sync.dma_start(hist[:, 1, :, 0], startc[128:256, :])

            score_bc = sbcpool.tile([128, BC, T], f32, tag="sbc")
            paths_sb = opool.tile([BC, s_len], i32, tag="po")
            bs_sb = opool.tile([BC, 1], f32, tag="bo")

            # ---------------- forward ----------------
            with (
                tc.tile_pool(name="psE", bufs=2, space="PSUM") as psE,
                tc.tile_pool(name="psR", bufs=2, space="PSUM") as psR,
                tc.tile_pool(name="psS", bufs=3, space="PSUM") as psS,
            ):
                for t in range(s_len - 1):
                    e_t = epool.tile([BC, T], f32, tag="e")
                    nc.sync.dma_start(e_t[:], em[:, t, :])
                    # transpose emissions to columns: ecol[i(128), jt, b]
                    ecol = ecpool.tile([128, 2, BC], f32, tag="ec")
                    eP = psE.tile([128, 2, BC], f32, tag="eP")
                    for jt in (0, 1):
                        nc.tensor.matmul(eP[:, jt, :],
                                         e_t[:, jt * 128:(jt + 1) * 128],
                                         i16_t[:], start=True, stop=True)
                    nc.scalar.copy(ecol[:], eP[:])
                    for c in range(NCH):
                        bsl = slice(c * CH, (c + 1) * CH)
                        # scol = hist cols + emission cols (score_t columns)
                        scol = scpool.tile([128, 2, CH], f32, tag="sc")
                        nc.vector.tensor_add(scol[:], hist[:, :, bsl, t],
                                             ecol[:, :, bsl])
                        # transpose to rows [CH, 256]
                        srows = srpool.tile([CH, T], f32, tag="sr")
                        sP = psR.tile([CH, 2, 128], f32, tag="sP")
                        for jt in (0, 1):
                            nc.tensor.transpose(sP[:, jt, :], scol[:, jt, :],
                                                i128)
                        nc.scalar.copy(
                            srows.rearrange("c (j x) -> c j x", j=2)[:], sP[:])
                        # broadcast each row b across 128 partitions, then
                        # fused add+max custom DVE op per (b, j-half)
                        for k in range(CH):
                            b = c * CH + k
                            Sb = psS.tile([128, T], f32, tag="S")
                            nc.tensor.matmul(Sb[:], sel_t[:, b, :], srows[:],
                                             start=True, stop=True)
                            nc.scalar.copy(score_bc[:, b, :], Sb[:])
                            for jt, tt in ((0, tt0), (1, tt1)):
                                w = tmppool.tile([128, T], f32, tag="w")
                                nc.vector._custom_dve(
                                    addmax, out=w[:], in0=tt[:],
                                    in1=score_bc[:, b, :],
                                    accum_out=hist[:, jt, b, t + 1:t + 2])

            # ---------------- final step + backward ----------------
            with (
                tc.tile_pool(name="psC", bufs=2, space="PSUM") as psC,
                tc.tile_pool(name="psT", bufs=2, space="PSUM") as psT,
                tc.tile_pool(name="psG", bufs=2, space="PSUM") as psG,
            ):
                ohc_prev = None
                for t in range(s_len - 1, -1, -1):
                    e_t = epool.tile([BC, T], f32, tag="e")
                    nc.sync.dma_start(e_t[:], em[:, t, :])
                    C = psC.tile([BC, T], f32, tag="C")
                    if ohc_prev is not None:
                        nc.tensor.matmul(C[:], ohc_prev[0][:], tt0[:],
                                         start=True, stop=False)
                        nc.tensor.matmul(C[:], ohc_prev[1][:], tt1[:],
                                         start=False, stop=False)
                        first = False
                    else:
                        first = True
                    nc.tensor.matmul(C[:], hist[:, 0, :, t], il_t[:],
                                     start=first, stop=False)
                    nc.tensor.matmul(C[:], hist[:, 1, :, t], ir_t[:],
                                     start=False, stop=False)
                    nc.tensor.matmul(C[:], i16_t[:], e_t[:],
                                     start=False, stop=True)
                    m = smallpool.tile([BC, 1], f32, tag="m")
                    nc.vector.reduce_max(m[:], C[:], axis=AX.X)
                    if ohc_prev is None:
                        nc.vector.tensor_copy(bs_sb[:], m[:])
                    ohr = smallpool.tile([BC, T], f32, tag="ohr")
                    nc.vector.tensor_scalar(out=ohr[:], in0=C[:], scalar1=m[:],
                                            scalar2=None, op0=OP.is_equal)
                    oh0 = ohpool.tile([128, BC], f32, tag="oh0")
                    oh1 = ohpool.tile([128, BC], f32, tag="oh1")
                    P = psT.tile([128, 2, BC], f32, tag="P")
                    for oh, half in ((oh0, 0), (oh1, 1)):
                        nc.tensor.matmul(P[:, half, :],
                                         ohr[:, half * 128:(half + 1) * 128],
                                         i16_t[:], start=True, stop=True)
                        nc.scalar.copy(oh[:], P[:, half, :])
                    tg = psG.tile([BC, 1], f32, tag="tg")
                    nc.tensor.matmul(tg[:], oh0[:], io0[:],
                                     start=True, stop=False)
                    nc.tensor.matmul(tg[:], oh1[:], io1[:],
                                     start=False, stop=True)
                    nc.scalar.copy(paths_sb[:, t:t + 1], tg[:])
                    ohc_prev = (oh0, oh1)

            nc.sync.dma_start(paths_d[:], paths_sb[:])
            nc.sync.dma_start(bs_d[:], bs_sb[:])

    nc.compile()
    return nc


def _host_inputs(emissions, transitions, start_transitions, end_transitions,
                 s_len):
    emissions = np.asarray(emissions, dtype=np.float32)
    transitions = np.asarray(transitions, dtype=np.float32)
    start_transitions = np.asarray(start_transitions, dtype=np.float32)
    end_transitions = np.asarray(end_transitions, dtype=np.float32)

    transTh = np.ascontiguousarray(transitions.T.astype(np.float32))
    startc = np.ascontiguousarray(
        np.tile(start_transitions[:, None], (1, BC)).astype(np.float32))
    ileft = np.zeros((128, T), np.float32)
    ileft[:, 0:128] = np.eye(128, dtype=np.float32)
    iright = np.zeros((128, T), np.float32)
    iright[:, 128:256] = np.eye(128, dtype=np.float32)
    i16 = np.eye(BC, dtype=np.float32)
    sel8 = np.zeros((CH, BC, 128), np.float32)
    for b in range(BC):
        sel8[b % CH, b, :] = 1.0
    iotac = np.arange(T, dtype=np.float32).reshape(T, 1)

    in_maps = []
    for c in range(NCORES):
        emc = np.array(emissions[c * BC:(c + 1) * BC, :s_len, :],
                       dtype=np.float32, copy=True)
        emc[:, s_len - 1, :] += end_transitions[None, :]
        in_maps.append({
            "em": emc, "transT": transTh, "startc": startc,
            "ileft": ileft, "iright": iright, "i16": i16,
            "sel8": sel8, "iotac": iotac,
        })
    return in_maps


def run(emissions, transitions, start_transitions, end_transitions,
        s_len=S, trace=False, n_cores=NCORES):
    from concourse.bass_utils import run_bass_kernel_spmd

    if s_len not in _compiled:
        _compiled[s_len] = _build(s_len)
    nc = _compiled[s_len]
    in_maps = _host_inputs(emissions, transitions, start_transitions,
                           end_transitions, s_len)[:n_cores]
    res = run_bass_kernel_spmd(nc, in_maps, core_ids=list(range(n_cores)),
                               trace=trace)
    paths = np.concatenate([res.results[c]["paths"] for c in range(n_cores)], 0)
    bs = np.concatenate([res.results[c]["bs"][:, 0] for c in range(n_cores)], 0)
    return paths.astype(np.int32), bs.astype(np.float32), res


def kernel(emissions, mask, transitions, start_transitions, end_transitions):
    paths, bs, _ = run(emissions, transitions, start_transitions,
                       end_transitions, s_len=S)
    return paths, bs
